# revision 35
# baseline (speedup 1.0000x reference)
"""Trainium2 Bass kernel for nn_DemandRouter (retrieval kNN).

Reference computation (per batch b):
    Q = x @ Wq.T + bq          [T, 32]
    K = x @ Wk.T + bk          [T, 32]
    sim = Q @ K.T / sqrt(32)   [T, T]
    idx = top_k(sim, 4)        [T, 4]
    out[t] = mean(x[idx[t]])   [T, D]

Sharding: 8 cores = 4 batches x 2 T-halves (data parallel over B, then
split the query rows T; every core projects keys for all T of its
batch). Each core receives x[b] ROLLED so its own 1024 query rows come
first — sim columns, top-k indices and the gather table all live in the
same rolled coordinate system, so the program is identical across cores
(SPMD) with no on-device offsets.

Measured HW slope: 39.2 us/iter/core (from the 66-80 us baseline).
Architecture (USE_OVL streaming emitter, the default):

  - ONE shared set of rotating tile pools for ALL repeat iterations.
    Closing a tile pool emits a release boundary that acts as a
    cross-engine barrier when the space is reused — per-iteration pool
    scopes serialized iterations (~90us stalls). With shared pools,
    iteration r+1's phase A/B overlaps iteration r's phase C/D through
    per-buffer deps only, and the software-pipelined emission places
    r+1's projection quads between r's C/D tiles so the in-order PE
    queue (exec window 32) can fill its scan-wait gaps.
  - Value path fp16 end to end: gather table (host pre-scaled by 0.25)
    and output store are float16 — halves gather traffic (16->8 MiB)
    and out traffic (4->2 MiB); rel err ~4e-4 vs the 2e-2 gate. The
    host upcasts the fp16 output to fp32.
  - SELECTION path stays effectively exact: phase A projects in fp32
    (f32r/fp16 projection flips top-4 near-ties: f32r measured 0.025
    rel err — rejected); the sim matmul splits the fp32 Q/K into fp16
    hi/lo pairs and computes qh*kh + qh*kl + ql*kh as ONE contract-96
    fp16 matmul (PE 1 cyc/col vs fp32's 4; ~2^-21 effective, measured
    0 flips over all 8192 rows).
  - Half-split sim PSUM (USE_HSIM): two [128, 1024] 2-bank tiles with
    bufs=3 instead of one 4-bank [128, 2048] — the next tile's sim
    matmuls overlap the current tile's DVE scans. Top-4 of a row lives
    in the union of the halves' top-8s (top-k in top-k blocks); global
    index = (ixh1 + 1024) min ixh0 in one scalar_tensor_tensor (the
    max_index not-found sentinel -1 wraps to 1023 but can never win
    the min when the value was found in h0). Projection PSUM is packed
    two groups per bank via partition-offset matmul outputs (2 banks),
    so PSUM = 2 + 3*2 = 8 banks.
  - Gathers: 4 single-index indirect DMAs per tile, cce-accumulated in
    pairs + one DVE add. Multi-index gathers are broken on this runtime
    in BOTH AP forms: out=[P,2,D] crashes (INTERNAL, bisected) and the
    flat out=[P,2*D] silently produces NaN garbage. Chaining all 4 into
    one tile via cce serializes the SBUF RMW (69.7 us) — all rejected.
    DMA accum on stores is SWDGE/gpsimd-only, so folding the mean-add
    into the store just moves work back onto Pool — rejected.
  - Queue placement: xrt loads + consts on SP (drains early, never
    gated by stores), phase-B activations + fp16-hi casts + output
    stores on ACT, hi/lo residual subtractions on gpsimd/Pool, scans +
    mean add on DVE, indirect gathers on Pool (SWDGE is gpsimd-only).
  - ~4us of dummy matmuls at kernel start ramp the PE p-state
    (0.65/1.2/2.4 GHz, >3us continuous busy for full clock).

The legacy per-iteration emitters (_emit_solo/_emit_pair) are kept
behind KERNEL_OVL=0 / KERNEL_PAIR=1 for A/B testing only.
"""

import os

import numpy as np

import concourse.bass as bass
import concourse.mybir as mybir
import concourse.tile as tile
from concourse import bacc
from concourse.bass import ts
from concourse.bass_utils import run_bass_kernel_spmd

B, T, D = 4, 2048, 1024
KQ = 32          # query/key projection width
KTOP = 4
P = 128
N_CORES = 8
TQ = T // 2      # query rows handled per core
ND = D // P      # 8 contraction chunks of 128
NG = 4           # t column-groups of full T
GT = T // NG     # 512 t per group
NGH = 2          # t column-groups of own half
NT = TQ // P     # 8 query row-tiles per core

f32 = mybir.dt.float32
f32r = mybir.dt.float32r
f16 = mybir.dt.float16
u32 = mybir.dt.uint32
IDENT = mybir.ActivationFunctionType.Identity

# experiment flags (read at module build time)
USE_F32R = os.environ.get("KERNEL_F32R", "0") == "1"
USE_CCE = os.environ.get("KERNEL_CCE", "1") == "1"
USE_PAIR = os.environ.get("KERNEL_PAIR", "0") == "1"
ABLATE = os.environ.get("KERNEL_ABLATE", "")
# fused 2-index gathers + stores on the ACT HWDGE ring + deeper pools
USE_V2CD = os.environ.get("KERNEL_V2CD", "1") == "1"
# fused 2-index gathers crash the runtime (INTERNAL error, bisected
# 2026-08-09); 4 single-index cce gathers in pairs are the working path.
USE_GIDX2 = os.environ.get("KERNEL_GIDX2", "0") == "1"
# cross-iteration pipelining: psim bufs=1 (sim 4 banks + proj 4 banks = 8)
# so iter r+1's phase A/B overlaps iter r's phase C/D; loads all on the SP
# queue (drains early), stores issued from the DVE queue (data-ready at
# issue, doesn't gate next iteration's loads).
USE_OVL = os.environ.get("KERNEL_OVL", "1") == "1"
# output stores via SWDGE on the Pool queue (else ACT HWDGE). Pool is
# ~76% busy with gather descriptor generation — stores go to ACT.
USE_PSTORE = os.environ.get("KERNEL_PSTORE", "0") == "1"
# pack hi/lo subtractions on gpsimd (keeps them off the DVE scan backbone)
USE_GSUB = os.environ.get("KERNEL_GSUB", "1") == "1"
# keep-warm dummy matmuls per C/D tile (0 disables; needs the packed
# 2-bank projection PSUM to free a bank). Modeled: no benefit — off.
N_WARM = int(os.environ.get("KERNEL_WARM", "0"))
# half-split sim tiles [128, 1024] (2 banks, bufs=3): the next tile's sim
# matmuls overlap the current tile's DVE scans. Top-4 of a row lives in
# the union of the halves' top-8s; global index = (ixh1+1024) min ixh0
# (max_index writes -1 for not-found; the u32 wrap to 1023 can never win
# the min when the value was found in h0).
USE_HSIM = os.environ.get("KERNEL_HSIM", "1") == "1"
TH = T // 2
# next-iteration projection-quad emission schedule per C/D tile
# (KERNEL_QLEAD=1: c2/c3 doubled at tiles 4-5; 0: one quad per tile)
if os.environ.get("KERNEL_QLEAD", "1") == "1":
    QSCHED = {0: [(0, 0)], 1: [(0, 1)], 2: [(1, 0)], 3: [(1, 1)],
              4: [(2, 0), (2, 1)], 5: [(3, 0), (3, 1)], 6: [], 7: []}
else:
    QSCHED = {i: [divmod(i, 2)] for i in range(8)}
# gather accumulation: 1 = all 4 cce-chained into one tile (no DVE add;
# modeled faster but slower on HW — sequential RMW), 0 = two cce pairs
# + one DVE add.
USE_GCHAIN = os.environ.get("KERNEL_GCHAIN", "0") == "1"
# fused multi-index gathers with a FLAT 2-D out AP [P, n*D] (the 3-D
# [P, n, D] form crashes the runtime): n = 2 or 4 indices per SWDGE call
GFLAT = int(os.environ.get("KERNEL_GFLAT", "0"))
# fold the pair-mean add into the output store (second store uses DMA
# accum_op=add on DRAM) — removes the last per-tile DVE op
USE_SADD = os.environ.get("KERNEL_SADD", "0") == "1"
# fp16 VALUE path: gather table + output store in float16 (selection path
# stays exact fp32). Halves gather traffic (16->8 MiB) and out traffic
# (4->2 MiB); emulated rel err ~5e-4, far under the 2e-2 gate.
USE_F16G = os.environ.get("KERNEL_F16G", "1") == "1"
# packed fp16 sim: split the exact fp32 Q/K into fp16 hi/lo pairs and
# compute sim = qh*kh + qh*kl + ql*kh as ONE contract-96 fp16 matmul
# (PE: 1 cyc/col vs fp32's 4). Effective ~2^-21 sim precision — measured
# 0 top-4 flips over all 8192 rows (f32r's ~2^-13 flipped enough for
# 0.025 rel err; this is 256x finer).
USE_PACK = os.environ.get("KERNEL_PACK", "1") == "1"

# float32r is *rounded* fp32 (reduced precision) — measured 0.025 rel err
# on this problem, so it stays off; exact fp32 everywhere.
MM_DT = f32r if USE_F32R else f32
# value-path dtype (gather table, gather tiles, mean, output store)
VDT = f16 if USE_F16G else f32
VNP = np.float16 if USE_F16G else np.float32

PAIR_GROUPS = [[0, 1], [2, 3], [4, 5], [6, 7]]

_NC = None


def _emit_warmup(tc, nc):
    from contextlib import ExitStack

    # ~4us of dummy matmuls so the PE p-state ramps to 2.4 GHz while the
    # first input DMA is in flight. Pools scoped so the PSUM bank frees
    # before phase C needs all 8.
    with ExitStack() as wctx:
        wu = wctx.enter_context(tc.tile_pool(name="wu", bufs=1))
        wups = wctx.enter_context(tc.tile_pool(name="wups", bufs=1, space="PSUM"))
        wsb = wu.tile([P, P], f32)
        nc.gpsimd.memset(wsb[:], 1.0)
        wps = wups.tile([P, P], f32)
        for _ in range(10):
            nc.tensor.matmul(wps[:], lhsT=wsb[:], rhs=wsb[:], start=True, stop=True)


def _emit_qk_pack(tc, nc, cpool, qt, kt, tq):
    """Split exact fp32 Q^T/K^T into fp16 hi/lo and build the packed
    contract-96 sim operands qtp=[qh;qh;ql], ktp=[kh;kl;kh] so that
    qtp.T @ ktp = qh*kh + qh*kl + ql*kh (the ll term ~2^-22 is dropped)."""
    qtp = cpool.tile([3 * KQ, tq], f16)
    ktp = cpool.tile([3 * KQ, T], f16)
    nc.scalar.activation(qtp[0:KQ, :], qt[:, 0:tq], IDENT)
    nc.scalar.activation(qtp[KQ : 2 * KQ, :], qt[:, 0:tq], IDENT)
    nc.vector.tensor_sub(qtp[2 * KQ : 3 * KQ, :], qt[:, 0:tq], qtp[0:KQ, :])
    nc.scalar.activation(ktp[0:KQ, :], kt[:], IDENT)
    nc.scalar.activation(ktp[2 * KQ : 3 * KQ, :], kt[:], IDENT)
    nc.vector.tensor_sub(ktp[KQ : 2 * KQ, :], kt[:], ktp[0:KQ, :])
    return qtp, ktp


def _emit_topk_gather(tc, nc, pcd, qt, kt, xg, out):
    """Phases C+D: sim, top-k, gather, mean, store."""
    psim = pcd.enter_context(
        tc.tile_pool(name="psim", bufs=1 if USE_OVL else 2, space="PSUM")
    )
    gpool = pcd.enter_context(tc.tile_pool(name="gpool", bufs=4 if USE_V2CD else 2))
    mpool = pcd.enter_context(tc.tile_pool(name="mpool", bufs=3))
    opool = pcd.enter_context(tc.tile_pool(name="opool", bufs=3 if USE_V2CD else 2))

    for i in range(NT):
        simp = psim.tile([P, T], f32, tag="sim", name=f"sim{i}")
        for c in range(NG):
            nc.tensor.matmul(
                simp[:, ts(c, GT)],
                lhsT=qt[:, ts(i, P)],
                rhs=kt[:, ts(c, GT)],
                start=True,
                stop=True,
            )
        mx = mpool.tile([P, 8], f32, tag="mx", name=f"mx{i}")
        ix = mpool.tile([P, 8], u32, tag="ix", name=f"ix{i}")
        nc.vector.max(out=mx[:], in_=simp[:])
        nc.vector.max_index(out=ix[:], in_max=mx[:], in_values=simp[:])

        if ABLATE == "nogather":
            g = [
                gpool.tile([P, D], VDT, tag=f"g{k}", name=f"g{k}_{i}")
                for k in range(2)
            ]
            nc.gpsimd.memset(g[0][:], 0.5)
            nc.gpsimd.memset(g[1][:], 0.25)
            s01 = opool.tile([P, D], VDT, tag="s01", name=f"s01_{i}")
            nc.vector.tensor_add(s01[:], g[0][:], g[1][:])
        elif USE_V2CD and USE_GIDX2:
            # two fused 2-index gathers: ga[p, j, :] = xg[ix[p, j]] then
            # += xg[ix[p, j+2]] via cce add; one DVE add folds j=0,1.
            ga = gpool.tile([P, 2, D], VDT, tag="ga", name=f"ga_{i}")
            nc.gpsimd.indirect_dma_start(
                out=ga[:],
                out_offset=None,
                in_=xg[:, :],
                in_offset=bass.IndirectOffsetOnAxis(ap=ix[:, 0:2], axis=0),
            )
            nc.gpsimd.indirect_dma_start(
                out=ga[:],
                out_offset=None,
                in_=xg[:, :],
                in_offset=bass.IndirectOffsetOnAxis(ap=ix[:, 2:4], axis=0),
                compute_op=mybir.AluOpType.add,
            )
            s01 = opool.tile([P, D], VDT, tag="s01", name=f"s01_{i}")
            nc.vector.tensor_add(s01[:], ga[:, 0, :], ga[:, 1, :])
        elif USE_CCE:
            g = [
                gpool.tile([P, D], VDT, tag=f"g{k}", name=f"g{k}_{i}")
                for k in range(2)
            ]
            for k in range(KTOP):
                nc.gpsimd.indirect_dma_start(
                    out=g[k % 2][:],
                    out_offset=None,
                    in_=xg[:, :],
                    in_offset=bass.IndirectOffsetOnAxis(ap=ix[:, k : k + 1], axis=0),
                    compute_op=(
                        mybir.AluOpType.add if k >= 2 else mybir.AluOpType.bypass
                    ),
                )
            s01 = opool.tile([P, D], VDT, tag="s01", name=f"s01_{i}")
            nc.vector.tensor_add(s01[:], g[0][:], g[1][:])
        else:
            g = [
                gpool.tile([P, D], VDT, tag=f"g{k}", name=f"g{k}_{i}")
                for k in range(KTOP)
            ]
            for k in range(KTOP):
                nc.gpsimd.indirect_dma_start(
                    out=g[k][:],
                    out_offset=None,
                    in_=xg[:, :],
                    in_offset=bass.IndirectOffsetOnAxis(ap=ix[:, k : k + 1], axis=0),
                )
            s01 = opool.tile([P, D], VDT, tag="s01", name=f"s01_{i}")
            s23 = opool.tile([P, D], VDT, tag="s23", name=f"s23_{i}")
            nc.vector.tensor_add(s01[:], g[0][:], g[1][:])
            nc.vector.tensor_add(s23[:], g[2][:], g[3][:])
            nc.vector.tensor_add(s01[:], s01[:], s23[:])
        # xg rows are pre-scaled by 0.25 on the host (exact power-of-two
        # scale), so s01 already is the 4-neighbor mean. Stores alternate
        # between the ACT and SP HWDGE rings (SP is idle during C/D).
        if USE_OVL:
            # SWDGE store on the Pool queue: issues right after this tile's
            # add (data-ready), never gates the next iteration's SP loads
            # or ACT phase-B work.
            nc.gpsimd.dma_start(out[ts(i, P), :], s01[:])
        elif USE_V2CD:
            seng = nc.scalar if i % 2 == 0 else nc.sync
            seng.dma_start(out[ts(i, P), :], s01[:])
        else:
            nc.sync.dma_start(out[ts(i, P), :], s01[:])


def _emit_pair(tc, nc, xg, xth, wqkt, bqk, out, warmup):
    """Pair-sharing variant: project own T-half only, AllGather K^T.

    Everything is in GLOBAL coordinates: sim columns are global t, the
    gather table xg is the unrolled x[b], and the output rows are the
    core's own global query rows.
    """
    from contextlib import ExitStack

    with ExitStack() as ctx:
        if warmup:
            _emit_warmup(tc, nc)
        cpool = ctx.enter_context(tc.tile_pool(name="consts", bufs=1))
        wq_sb = cpool.tile([P, ND, 2 * KQ], MM_DT)  # [128, 8, 64]; d = dd*128+p
        nc.sync.dma_start(wq_sb[:], wqkt.rearrange("(n p) k -> p n k", p=P))
        bqk_sb = cpool.tile([2 * KQ, 1], f32)
        nc.sync.dma_start(bqk_sb[:], bqk[:])
        qt = cpool.tile([KQ, TQ], f32)  # Q^T (own half) with bias
        kt = cpool.tile([KQ, NGH, TQ], f32)  # K^T (full T) with bias

        dpool = ctx.enter_context(tc.tile_pool(name="ccdram", bufs=1, space="DRAM"))
        cc_in = dpool.tile([KQ, TQ], f32)
        cc_out = dpool.tile([2 * KQ, TQ], f32)

        # ---- phase A: load own xth half + project ----
        with ExitStack() as pa:
            xt_pool = pa.enter_context(tc.tile_pool(name="xt", bufs=3))
            pqkt = pa.enter_context(tc.tile_pool(name="pqkt", bufs=1, space="PSUM"))
            qk_ps = [
                pqkt.tile([2 * KQ, GT], f32, tag=f"qk{c}", name=f"qk_ps{c}")
                for c in range(NGH)
            ]
            kth = cpool.tile([KQ, TQ], f32)  # own biased K^T half
            if ABLATE == "noproj":
                nc.vector.memset(qt[:], 0.001)
                nc.vector.memset(kth[:], 0.002)
            for dd in range(ND if ABLATE != "noproj" else 0):
                xt = xt_pool.tile([P, TQ], MM_DT, tag="xt", name=f"xt{dd}")
                nc.sync.dma_start(xt[:], xth[ts(dd, P), :])
                for c in range(NGH):
                    nc.tensor.matmul(
                        qk_ps[c][:],
                        lhsT=wq_sb[:, dd, :],
                        rhs=xt[:, ts(c, GT)],
                        start=(dd == 0),
                        stop=(dd == ND - 1),
                    )

            # ---- phase B: PSUM -> SBUF with bias ----
            for c in range(NGH if ABLATE != "noproj" else 0):
                nc.scalar.activation(
                    qt[:, ts(c, GT)], qk_ps[c][0:KQ, :], IDENT, bias=bqk_sb[0:KQ, :]
                )
                nc.scalar.activation(
                    kth[:, ts(c, GT)],
                    qk_ps[c][KQ : 2 * KQ, :],
                    IDENT,
                    bias=bqk_sb[KQ : 2 * KQ, :],
                )
        nc.sync.dma_start(cc_in[:], kth[:])
        nc.gpsimd.collective_compute(
            "AllGather",
            mybir.AluOpType.bypass,
            replica_groups=PAIR_GROUPS,
            ins=[cc_in[:]],
            outs=[cc_out[:]],
        )
        # cc_out rows [0:32] = pair rank 0 (global t 0..1023), rows
        # [32:64] = pair rank 1 — global column order for both cores.
        nc.sync.dma_start(kt[:], cc_out.rearrange("(h k) s -> k h s", k=KQ))

        with ExitStack() as pcd:
            ktf = kt.rearrange("k h s -> k (h s)")
            if USE_PACK:
                qt, ktf = _emit_qk_pack(tc, nc, cpool, qt, ktf, TQ)
            _emit_topk_gather(tc, nc, pcd, qt, ktf, xg, out)


def _emit_solo(tc, nc, xg, xrt, wqkt, bqk, out, warmup):
    """Original variant: every core projects all T keys itself (rolled
    coordinates: the core's queries are rows [0:1024) of the rolled x)."""
    from contextlib import ExitStack

    with ExitStack() as ctx:
        if warmup:
            _emit_warmup(tc, nc)
        cpool = ctx.enter_context(tc.tile_pool(name="consts", bufs=1))
        wq_sb = cpool.tile([P, ND, 2 * KQ], MM_DT)
        nc.sync.dma_start(wq_sb[:], wqkt.rearrange("(n p) k -> p n k", p=P))
        bqk_sb = cpool.tile([2 * KQ, 1], f32)
        nc.sync.dma_start(bqk_sb[:], bqk[:])
        qt = cpool.tile([KQ, T], f32)
        kt = cpool.tile([KQ, T], f32)

        with ExitStack() as pa:
            xt_pool = pa.enter_context(tc.tile_pool(name="xt", bufs=3))
            pqkt = pa.enter_context(tc.tile_pool(name="pqkt", bufs=1, space="PSUM"))
            qk_ps = [
                pqkt.tile([2 * KQ, GT], f32, tag=f"qk{c}", name=f"qk_ps{c}")
                for c in range(NG)
            ]
            if ABLATE == "noproj":
                nc.vector.memset(qt[:], 0.001)
                nc.vector.memset(kt[:], 0.002)
            for dd in range(ND if ABLATE != "noproj" else 0):
                xt = xt_pool.tile([P, T], MM_DT, tag="xt", name=f"xt{dd}")
                if USE_OVL:
                    eng = nc.sync  # SP queue only: drains before r+1 needs it
                else:
                    # alternate load issue across both HWDGE rings (SP/ACT)
                    eng = nc.sync if (dd % 2 == 0 or not USE_V2CD) else nc.scalar
                eng.dma_start(xt[:], xrt[ts(dd, P), :])
                for c in range(NG):
                    nc.tensor.matmul(
                        qk_ps[c][:],
                        lhsT=wq_sb[:, dd, :],
                        rhs=xt[:, ts(c, GT)],
                        start=(dd == 0),
                        stop=(dd == ND - 1),
                    )
            for c in range(NG if ABLATE != "noproj" else 0):
                nc.scalar.activation(
                    qt[:, ts(c, GT)], qk_ps[c][0:KQ, :], IDENT, bias=bqk_sb[0:KQ, :]
                )
                nc.scalar.activation(
                    kt[:, ts(c, GT)],
                    qk_ps[c][KQ : 2 * KQ, :],
                    IDENT,
                    bias=bqk_sb[KQ : 2 * KQ, :],
                )

        if USE_PACK:
            qt, kt = _emit_qk_pack(tc, nc, cpool, qt, kt, TQ)
        with ExitStack() as pcd:
            _emit_topk_gather(tc, nc, pcd, qt, kt, xg, out)


def _emit_stream(tc, nc, xg, xrt, wqkt, bqk, out, repeat):
    """All `repeat` iterations emitted into ONE set of shared pools, with
    iteration r+1's phase A/B interleaved between iteration r's C/D tiles.

    Why: (1) closing a tile pool emits a release boundary that any later
    pool reusing the space waits on across all engines — per-iteration
    pool scopes act as inter-iteration barriers (measured ~90us stall).
    (2) Engine queues are strictly in-order, so iteration r+1's phase-A
    matmuls can only fill the PE idle gaps between iteration r's sim
    bursts if they are EMITTED between them. The interleave also keeps
    PE continuously busy so its p-state stays at 2.4 GHz (idle resets
    the clock ramp; cold fp32 matmuls run ~3x slower).

    Schedule: phase A runs group-major (each group's 8-chunk PSUM
    accumulation completes early) split into half-group quads — C/D tile
    i of iteration r is followed by quad (c=i//2, half=i%2) of iteration
    r+1, and by phase B for group c after the second half. All 8 xt
    chunks are SBUF-resident (bufs=2 for cross-iteration rotation).

    PSUM budget: qk_ps 4 banks + sim 4 banks = 8. Queues: loads+consts
    on SP, phase-B activations on ACT, pack subs on Pool (gpsimd) to
    keep them off the DVE scan backbone, sims+A-quads on PE, scans and
    the mean add on DVE, gathers and output stores on Pool/SWDGE.
    """
    from contextlib import ExitStack

    with ExitStack() as ctx:
        _emit_warmup(tc, nc)
        cpool = ctx.enter_context(tc.tile_pool(name="consts", bufs=1))
        wq_sb = cpool.tile([P, ND, 2 * KQ], MM_DT)
        nc.sync.dma_start(wq_sb[:], wqkt.rearrange("(n p) k -> p n k", p=P))
        bqk_sb = cpool.tile([2 * KQ, 1], f32)
        nc.sync.dma_start(bqk_sb[:], bqk[:])

        qkpool = ctx.enter_context(tc.tile_pool(name="qkp", bufs=2))
        xt_pool = ctx.enter_context(tc.tile_pool(name="xt", bufs=2))
        pqkt = ctx.enter_context(tc.tile_pool(name="pqkt", bufs=1, space="PSUM"))
        psim = ctx.enter_context(
            tc.tile_pool(name="psim", bufs=3 if USE_HSIM else 1, space="PSUM")
        )
        gpool = ctx.enter_context(tc.tile_pool(name="gpool", bufs=4))
        mpool = ctx.enter_context(tc.tile_pool(name="mpool", bufs=3))
        opool = ctx.enter_context(tc.tile_pool(name="opool", bufs=3))
        if N_WARM:
            # keep-warm: always-ready fp16 dummy matmuls into a dedicated
            # PSUM bank fill PE idle gaps so the p-state stays at 2.4 GHz
            # (PE clock decays on idle; cold bursts run ~2-3x slower).
            hotps = ctx.enter_context(tc.tile_pool(name="hotps", bufs=1, space="PSUM"))
            dmy = cpool.tile([P, GT], f16)
            nc.vector.memset(dmy[:], 0.001)
            hot = hotps.tile([P, GT], f32)

            def emit_warm(n):
                for _ in range(n):
                    nc.tensor.matmul(
                        hot[:], lhsT=dmy[:, 0:P], rhs=dmy[:], start=True, stop=True
                    )
        else:
            def emit_warm(n):
                pass

        def alloc_iter(r):
            return dict(
                xts=[
                    xt_pool.tile([P, T], MM_DT, tag=f"xt{dd}", name=f"xt{dd}_{r}")
                    for dd in range(ND)
                ],
                ps=[
                    pqkt.tile([2 * P // 2, GT], f32, tag=f"qk{j}", name=f"qk_ps{j}_{r}")
                    for j in range(NG // 2)
                ],
                qt=qkpool.tile([KQ, TQ], f32, tag="qt", name=f"qt_{r}"),
                kt=qkpool.tile([KQ, T], f32, tag="kt", name=f"kt_{r}"),
                qtp=qkpool.tile([3 * KQ, TQ], f16, tag="qtp", name=f"qtp_{r}"),
                ktp=qkpool.tile([3 * KQ, T], f16, tag="ktp", name=f"ktp_{r}"),
            )

        def emit_loads(it):
            for dd in range(ND):
                nc.sync.dma_start(it["xts"][dd][:], xrt[ts(dd, P), :])

        def emit_A_quad(it, c, half):
            j, sub = divmod(c, 2)
            dst = it["ps"][j][sub * 2 * KQ : (sub + 1) * 2 * KQ, :]
            for dd in range(4 * half, 4 * half + 4):
                nc.tensor.matmul(
                    dst,
                    lhsT=wq_sb[:, dd, :],
                    rhs=it["xts"][dd][:, ts(c, GT)],
                    start=(dd == 0),
                    stop=(dd == ND - 1),
                )

        def emit_B_group(it, c):
            qt, kt, qtp, ktp = it["qt"], it["kt"], it["qtp"], it["ktp"]
            sl = ts(c, GT)
            sub = nc.gpsimd.tensor_sub if USE_GSUB else nc.vector.tensor_sub
            j, psub = divmod(c, 2)
            base = psub * 2 * KQ
            if c < NGH:
                nc.scalar.activation(
                    qt[:, sl],
                    it["ps"][j][base : base + KQ, :],
                    IDENT,
                    bias=bqk_sb[0:KQ, :],
                )
            nc.scalar.activation(
                kt[:, sl],
                it["ps"][j][base + KQ : base + 2 * KQ, :],
                IDENT,
                bias=bqk_sb[KQ : 2 * KQ, :],
            )
            if c < NGH:
                nc.scalar.activation(qtp[0:KQ, sl], qt[:, sl], IDENT)
                nc.scalar.activation(qtp[KQ : 2 * KQ, sl], qt[:, sl], IDENT)
                sub(qtp[2 * KQ : 3 * KQ, sl], qt[:, sl], qtp[0:KQ, sl])
            nc.scalar.activation(ktp[0:KQ, sl], kt[:, sl], IDENT)
            nc.scalar.activation(ktp[2 * KQ : 3 * KQ, sl], kt[:, sl], IDENT)
            sub(ktp[KQ : 2 * KQ, sl], kt[:, sl], ktp[0:KQ, sl])

        def emit_CD_tile(it, r, i):
            qtp, ktp = it["qtp"], it["ktp"]
            if USE_HSIM:
                simh = [
                    psim.tile([P, TH], f32, tag="sim", name=f"sim{i}h{h}_{r}")
                    for h in range(2)
                ]
                for h in range(2):
                    for c in range(2):
                        g = 2 * h + c
                        nc.tensor.matmul(
                            simh[h][:, ts(c, GT)],
                            lhsT=qtp[:, ts(i, P)],
                            rhs=ktp[:, ts(g, GT)],
                            start=True,
                            stop=True,
                        )
                mxh = mpool.tile([P, 2, 8], f32, tag="mxh", name=f"mxh{i}_{r}")
                nc.vector.max(out=mxh[:, 0, :], in_=simh[0][:])
                nc.vector.max(out=mxh[:, 1, :], in_=simh[1][:])
                vm = mpool.tile([P, 8], f32, tag="vm", name=f"vm{i}_{r}")
                nc.vector.max(out=vm[:], in_=mxh[:])
                ixh0 = mpool.tile([P, 8], u32, tag="ixh0", name=f"ixh0_{i}_{r}")
                ixh1 = mpool.tile([P, 8], u32, tag="ixh1", name=f"ixh1_{i}_{r}")
                nc.vector.max_index(out=ixh0[:], in_max=vm[:], in_values=simh[0][:])
                nc.vector.max_index(out=ixh1[:], in_max=vm[:], in_values=simh[1][:])
                ix = mpool.tile([P, 8], u32, tag="ix", name=f"ix{i}_{r}")
                nc.vector.scalar_tensor_tensor(
                    out=ix[:],
                    in0=ixh1[:],
                    scalar=TH,
                    in1=ixh0[:],
                    op0=mybir.AluOpType.add,
                    op1=mybir.AluOpType.min,
                )
            else:
                simp = psim.tile([P, T], f32, tag="sim", name=f"sim{i}_{r}")
                for c in range(NG):
                    nc.tensor.matmul(
                        simp[:, ts(c, GT)],
                        lhsT=qtp[:, ts(i, P)],
                        rhs=ktp[:, ts(c, GT)],
                        start=True,
                        stop=True,
                    )
                mx = mpool.tile([P, 8], f32, tag="mx", name=f"mx{i}_{r}")
                ix = mpool.tile([P, 8], u32, tag="ix", name=f"ix{i}_{r}")
                nc.vector.max(out=mx[:], in_=simp[:])
                nc.vector.max_index(out=ix[:], in_max=mx[:], in_values=simp[:])

            if USE_GIDX2:
                ga = gpool.tile([P, 2, D], VDT, tag="ga", name=f"ga_{i}_{r}")
                nc.gpsimd.indirect_dma_start(
                    out=ga[:],
                    out_offset=None,
                    in_=xg[:, :],
                    in_offset=bass.IndirectOffsetOnAxis(ap=ix[:, 0:2], axis=0),
                )
                nc.gpsimd.indirect_dma_start(
                    out=ga[:],
                    out_offset=None,
                    in_=xg[:, :],
                    in_offset=bass.IndirectOffsetOnAxis(ap=ix[:, 2:4], axis=0),
                    compute_op=mybir.AluOpType.add,
                )
                a0, a1 = ga[:, 0, :], ga[:, 1, :]
                s01 = opool.tile([P, D], VDT, tag="s01", name=f"s01_{i}_{r}")
                nc.vector.tensor_add(s01[:], a0, a1)
                sv = s01[:]
            elif GFLAT == 4:
                gf = gpool.tile([P, 4 * D], VDT, tag="gf", name=f"gf_{i}_{r}")
                nc.gpsimd.indirect_dma_start(
                    out=gf[:],
                    out_offset=None,
                    in_=xg[:, :],
                    in_offset=bass.IndirectOffsetOnAxis(ap=ix[:, 0:4], axis=0),
                )
                s01 = opool.tile([P, D], VDT, tag="s01", name=f"s01_{i}_{r}")
                s23 = opool.tile([P, D], VDT, tag="s23", name=f"s23_{i}_{r}")
                nc.vector.tensor_add(s01[:], gf[:, 0:D], gf[:, D : 2 * D])
                nc.vector.tensor_add(s23[:], gf[:, 2 * D : 3 * D], gf[:, 3 * D :])
                nc.vector.tensor_add(s01[:], s01[:], s23[:])
                sv = s01[:]
            elif GFLAT == 2:
                gf = gpool.tile([P, 2 * D], VDT, tag="gf", name=f"gf_{i}_{r}")
                nc.gpsimd.indirect_dma_start(
                    out=gf[:],
                    out_offset=None,
                    in_=xg[:, :],
                    in_offset=bass.IndirectOffsetOnAxis(ap=ix[:, 0:2], axis=0),
                )
                nc.gpsimd.indirect_dma_start(
                    out=gf[:],
                    out_offset=None,
                    in_=xg[:, :],
                    in_offset=bass.IndirectOffsetOnAxis(ap=ix[:, 2:4], axis=0),
                    compute_op=mybir.AluOpType.add,
                )
                if USE_SADD:
                    nc.gpsimd.dma_start(out[ts(i, P), :], gf[:, 0:D])
                    nc.gpsimd.dma_start(
                        out[ts(i, P), :], gf[:, D:], accum_op=mybir.AluOpType.add
                    )
                    return
                s01 = opool.tile([P, D], VDT, tag="s01", name=f"s01_{i}_{r}")
                nc.vector.tensor_add(s01[:], gf[:, 0:D], gf[:, D:])
                sv = s01[:]
            elif USE_GCHAIN:
                # all four gathers cce-accumulate into ONE tile: the DMA
                # queue orders the adds, no DVE op needed — the host 0.25
                # pre-scale makes the accumulated tile the final mean.
                g0 = gpool.tile([P, D], VDT, tag="g0", name=f"g0_{i}_{r}")
                for k in range(KTOP):
                    nc.gpsimd.indirect_dma_start(
                        out=g0[:],
                        out_offset=None,
                        in_=xg[:, :],
                        in_offset=bass.IndirectOffsetOnAxis(
                            ap=ix[:, k : k + 1], axis=0
                        ),
                        compute_op=(
                            mybir.AluOpType.add if k >= 1 else mybir.AluOpType.bypass
                        ),
                    )
                sv = g0[:]
            else:
                g = [
                    gpool.tile([P, D], VDT, tag=f"g{k}", name=f"g{k}_{i}_{r}")
                    for k in range(2)
                ]
                for k in range(KTOP):
                    nc.gpsimd.indirect_dma_start(
                        out=g[k % 2][:],
                        out_offset=None,
                        in_=xg[:, :],
                        in_offset=bass.IndirectOffsetOnAxis(
                            ap=ix[:, k : k + 1], axis=0
                        ),
                        compute_op=(
                            mybir.AluOpType.add if k >= 2 else mybir.AluOpType.bypass
                        ),
                    )
                if USE_SADD:
                    nc.gpsimd.dma_start(out[ts(i, P), :], g[0][:])
                    nc.gpsimd.dma_start(
                        out[ts(i, P), :], g[1][:], accum_op=mybir.AluOpType.add
                    )
                    return
                s01 = opool.tile([P, D], VDT, tag="s01", name=f"s01_{i}_{r}")
                nc.vector.tensor_add(s01[:], g[0][:], g[1][:])
                sv = s01[:]
            if USE_PSTORE:
                nc.gpsimd.dma_start(out[ts(i, P), :], sv)
            else:
                nc.scalar.dma_start(out[ts(i, P), :], sv)

        # prologue: iteration 0's phase A + B run un-overlapped
        cur = alloc_iter(0)
        emit_loads(cur)
        for c in range(NG):
            emit_A_quad(cur, c, 0)
            emit_A_quad(cur, c, 1)
            emit_B_group(cur, c)
        for r in range(repeat):
            nxt = None
            if r + 1 < repeat:
                nxt = alloc_iter(r + 1)
                emit_loads(nxt)
            for i in range(NT):
                emit_CD_tile(cur, r, i)
                emit_warm(N_WARM)
                if nxt is not None:
                    # front-load the late groups: c2/c3 run doubled slots at
                    # tiles 4-5 (their chunks have all landed by then), so
                    # the full ktp pack completes ~2 tiles before the
                    # boundary instead of at it — the next iteration's sims
                    # were measured stalling ~17us on B(c3).
                    for c, half in QSCHED[i]:
                        emit_A_quad(nxt, c, half)
                        if half == 1:
                            emit_B_group(nxt, c)
            cur = nxt


def _build_module():
    repeat = int(os.environ.get("KERNEL_REPEAT", "1"))
    nc = bacc.Bacc(
        "TRN2", target_bir_lowering=False, debug=False, num_devices=N_CORES
    )
    if USE_PAIR:
        xg = nc.dram_tensor("xg", [T, D], VDT, kind="ExternalInput").ap()
        xth = nc.dram_tensor("xth", [D, TQ], MM_DT, kind="ExternalInput").ap()
        wqkt = nc.dram_tensor("wqkt", [D, 2 * KQ], MM_DT, kind="ExternalInput").ap()
        bqk = nc.dram_tensor("bqk", [2 * KQ, 1], f32, kind="ExternalInput").ap()
        out = nc.dram_tensor("out", [TQ, D], VDT, kind="ExternalOutput").ap()
        with tile.TileContext(nc) as tc:
            for r in range(repeat):
                _emit_pair(tc, nc, xg, xth, wqkt, bqk, out, warmup=(r == 0))
    else:
        xg = nc.dram_tensor("xr", [T, D], VDT, kind="ExternalInput").ap()
        xrt = nc.dram_tensor("xrt", [D, T], MM_DT, kind="ExternalInput").ap()
        wqkt = nc.dram_tensor("wqkt", [D, 2 * KQ], MM_DT, kind="ExternalInput").ap()
        bqk = nc.dram_tensor("bqk", [2 * KQ, 1], f32, kind="ExternalInput").ap()
        out = nc.dram_tensor("out", [TQ, D], VDT, kind="ExternalOutput").ap()
        with tile.TileContext(nc) as tc:
            if USE_OVL:
                _emit_stream(tc, nc, xg, xrt, wqkt, bqk, out, repeat)
            else:
                for r in range(repeat):
                    _emit_solo(tc, nc, xg, xrt, wqkt, bqk, out, warmup=(r == 0))
    nc.compile()
    return nc


def _get_nc():
    global _NC
    if _NC is None:
        _NC = _build_module()
    return _NC


def _make_in_maps(x, Wq, bq, Wk, bk):
    x = np.ascontiguousarray(np.asarray(x, dtype=np.float32))
    wqkt = np.ascontiguousarray(
        np.concatenate(
            [np.asarray(Wq, np.float32).T, np.asarray(Wk, np.float32).T], axis=1
        )
    )
    bqk = np.concatenate(
        [np.asarray(bq, np.float32), np.asarray(bk, np.float32)]
    )[:, None]
    bqk = np.ascontiguousarray(bqk)
    in_maps = []
    # exact (power of two) pre-scale; fp16 value-path cast adds ~4e-4 rel err
    xq = (x * np.float32(0.25)).astype(VNP)
    for c in range(N_CORES):
        b, h = divmod(c, 2)
        off = h * TQ
        xb = x[b]
        if USE_PAIR:
            in_maps.append(
                {
                    "xg": np.ascontiguousarray(xq[b]),
                    "xth": np.ascontiguousarray(xb[off : off + TQ].T),
                    "wqkt": wqkt,
                    "bqk": bqk,
                }
            )
        else:
            xrc = (
                np.concatenate([xq[b][off:], xq[b][:off]], axis=0)
                if off
                else xq[b]
            )
            in_maps.append(
                {
                    "xr": np.ascontiguousarray(xrc),
                    "xrt": np.ascontiguousarray(xb.T) if off == 0 else
                           np.ascontiguousarray(
                               np.concatenate([xb[off:], xb[:off]], axis=0).T),
                    "wqkt": wqkt,
                    "bqk": bqk,
                }
            )
    return in_maps


def run(x, Wq, bq, Wk, bk, trace=False):
    """Run on 8 cores; returns (full_output, BassKernelResults)."""
    in_maps = _make_in_maps(x, Wq, bq, Wk, bk)
    nc = _get_nc()
    res = run_bass_kernel_spmd(nc, in_maps, list(range(N_CORES)), trace=trace)
    outf = np.empty((B, T, D), np.float32)
    for c in range(N_CORES):
        b, h = divmod(c, 2)
        # device stores VDT (fp16); upcast to fp32 on assignment
        outf[b, h * TQ : (h + 1) * TQ] = res.results[c]["out"]
    return outf, res


def kernel(x, Wq, bq, Wk, bk):
    outf, _ = run(x, Wq, bq, Wk, bk, trace=False)
    return outf



# revision 38
# speedup vs baseline: 1.0327x; 1.0327x over previous
"""Trainium2 Bass kernel for nn_DemandRouter (retrieval kNN).

Reference computation (per batch b):
    Q = x @ Wq.T + bq          [T, 32]
    K = x @ Wk.T + bk          [T, 32]
    sim = Q @ K.T / sqrt(32)   [T, T]
    idx = top_k(sim, 4)        [T, 4]
    out[t] = mean(x[idx[t]])   [T, D]

Sharding: 8 cores = 4 batches x 2 T-halves (data parallel over B, then
split the query rows T; every core projects keys for all T of its
batch). Each core receives x[b] ROLLED so its own 1024 query rows come
first — sim columns, top-k indices and the gather table all live in the
same rolled coordinate system, so the program is identical across cores
(SPMD) with no on-device offsets.

Measured HW slope: 39.2 us/iter/core (from the 66-80 us baseline).
Architecture (USE_OVL streaming emitter, the default):

  - ONE shared set of rotating tile pools for ALL repeat iterations.
    Closing a tile pool emits a release boundary that acts as a
    cross-engine barrier when the space is reused — per-iteration pool
    scopes serialized iterations (~90us stalls). With shared pools,
    iteration r+1's phase A/B overlaps iteration r's phase C/D through
    per-buffer deps only, and the software-pipelined emission places
    r+1's projection quads between r's C/D tiles so the in-order PE
    queue (exec window 32) can fill its scan-wait gaps.
  - Value path fp16 end to end: gather table (host pre-scaled by 0.25)
    and output store are float16 — halves gather traffic (16->8 MiB)
    and out traffic (4->2 MiB); rel err ~4e-4 vs the 2e-2 gate. The
    host upcasts the fp16 output to fp32.
  - SELECTION path stays effectively exact: phase A projects in fp32
    (f32r/fp16 projection flips top-4 near-ties: f32r measured 0.025
    rel err — rejected); the sim matmul splits the fp32 Q/K into fp16
    hi/lo pairs and computes qh*kh + qh*kl + ql*kh as ONE contract-96
    fp16 matmul (PE 1 cyc/col vs fp32's 4; ~2^-21 effective, measured
    0 flips over all 8192 rows).
  - Half-split sim PSUM (USE_HSIM): two [128, 1024] 2-bank tiles with
    bufs=3 instead of one 4-bank [128, 2048] — the next tile's sim
    matmuls overlap the current tile's DVE scans. Top-4 of a row lives
    in the union of the halves' top-8s (top-k in top-k blocks); global
    index = (ixh1 + 1024) min ixh0 in one scalar_tensor_tensor (the
    max_index not-found sentinel -1 wraps to 1023 but can never win
    the min when the value was found in h0). Projection PSUM is packed
    two groups per bank via partition-offset matmul outputs (2 banks),
    so PSUM = 2 + 3*2 = 8 banks.
  - Gathers: 4 single-index indirect DMAs per tile, cce-accumulated in
    pairs + one DVE add. Multi-index gathers are broken on this runtime
    in BOTH AP forms: out=[P,2,D] crashes (INTERNAL, bisected) and the
    flat out=[P,2*D] silently produces NaN garbage. Chaining all 4 into
    one tile via cce serializes the SBUF RMW (69.7 us) — all rejected.
    DMA accum on stores is SWDGE/gpsimd-only, so folding the mean-add
    into the store just moves work back onto Pool — rejected.
  - Queue placement: xrt loads + consts on SP (drains early, never
    gated by stores), phase-B activations + fp16-hi casts + output
    stores on ACT, hi/lo residual subtractions on gpsimd/Pool, scans +
    mean add on DVE, indirect gathers on Pool (SWDGE is gpsimd-only).
  - ~4us of dummy matmuls at kernel start ramp the PE p-state
    (0.65/1.2/2.4 GHz, >3us continuous busy for full clock).

The legacy per-iteration emitters (_emit_solo/_emit_pair) are kept
behind KERNEL_OVL=0 / KERNEL_PAIR=1 for A/B testing only.
"""

import os

import numpy as np

import concourse.bass as bass
import concourse.mybir as mybir
import concourse.tile as tile
from concourse import bacc
from concourse.bass import ts
from concourse.bass_utils import run_bass_kernel_spmd

B, T, D = 4, 2048, 1024
KQ = 32          # query/key projection width
KTOP = 4
P = 128
N_CORES = 8
TQ = T // 2      # query rows handled per core
ND = D // P      # 8 contraction chunks of 128
NG = 4           # t column-groups of full T
GT = T // NG     # 512 t per group
NGH = 2          # t column-groups of own half
NT = TQ // P     # 8 query row-tiles per core

f32 = mybir.dt.float32
f32r = mybir.dt.float32r
f16 = mybir.dt.float16
u32 = mybir.dt.uint32
IDENT = mybir.ActivationFunctionType.Identity

# experiment flags (read at module build time)
USE_F32R = os.environ.get("KERNEL_F32R", "0") == "1"
USE_CCE = os.environ.get("KERNEL_CCE", "1") == "1"
USE_PAIR = os.environ.get("KERNEL_PAIR", "0") == "1"
ABLATE = os.environ.get("KERNEL_ABLATE", "")
# fused 2-index gathers + stores on the ACT HWDGE ring + deeper pools
USE_V2CD = os.environ.get("KERNEL_V2CD", "1") == "1"
# fused 2-index gathers crash the runtime (INTERNAL error, bisected
# 2026-08-09); 4 single-index cce gathers in pairs are the working path.
USE_GIDX2 = os.environ.get("KERNEL_GIDX2", "0") == "1"
# cross-iteration pipelining: psim bufs=1 (sim 4 banks + proj 4 banks = 8)
# so iter r+1's phase A/B overlaps iter r's phase C/D; loads all on the SP
# queue (drains early), stores issued from the DVE queue (data-ready at
# issue, doesn't gate next iteration's loads).
USE_OVL = os.environ.get("KERNEL_OVL", "1") == "1"
# output stores via SWDGE on the Pool queue (else ACT HWDGE). Pool is
# ~76% busy with gather descriptor generation — stores go to ACT.
USE_PSTORE = os.environ.get("KERNEL_PSTORE", "0") == "1"
# pack hi/lo subtractions on gpsimd (keeps them off the DVE scan backbone)
USE_GSUB = os.environ.get("KERNEL_GSUB", "1") == "1"
# keep-warm dummy matmuls per C/D tile (0 disables; needs the packed
# 2-bank projection PSUM to free a bank). Modeled: no benefit — off.
N_WARM = int(os.environ.get("KERNEL_WARM", "0"))
# half-split sim tiles [128, 1024] (2 banks, bufs=3): the next tile's sim
# matmuls overlap the current tile's DVE scans. Top-4 of a row lives in
# the union of the halves' top-8s; global index = (ixh1+1024) min ixh0
# (max_index writes -1 for not-found; the u32 wrap to 1023 can never win
# the min when the value was found in h0).
USE_HSIM = os.environ.get("KERNEL_HSIM", "1") == "1"
TH = T // 2
# next-iteration projection-quad emission schedule per C/D tile
# (KERNEL_QLEAD=1: c2/c3 doubled at tiles 4-5 to finish the pack ~2 tiles
# before the boundary; model-neutral, HW sampled 57.8 vs base 39-57 — off)
if os.environ.get("KERNEL_QLEAD", "0") == "1":
    QSCHED = {0: [(0, 0)], 1: [(0, 1)], 2: [(1, 0)], 3: [(1, 1)],
              4: [(2, 0), (2, 1)], 5: [(3, 0), (3, 1)], 6: [], 7: []}
else:
    QSCHED = {i: [divmod(i, 2)] for i in range(8)}
# gather accumulation: 1 = all 4 cce-chained into one tile (no DVE add;
# modeled faster but slower on HW — sequential RMW), 0 = two cce pairs
# + one DVE add.
USE_GCHAIN = os.environ.get("KERNEL_GCHAIN", "0") == "1"
# fused multi-index gathers with a FLAT 2-D out AP [P, n*D] (the 3-D
# [P, n, D] form crashes the runtime): n = 2 or 4 indices per SWDGE call
GFLAT = int(os.environ.get("KERNEL_GFLAT", "0"))
# fold the pair-mean add into the output store (second store uses DMA
# accum_op=add on DRAM) — removes the last per-tile DVE op
USE_SADD = os.environ.get("KERNEL_SADD", "0") == "1"
# fp16 VALUE path: gather table + output store in float16 (selection path
# stays exact fp32). Halves gather traffic (16->8 MiB) and out traffic
# (4->2 MiB); emulated rel err ~5e-4, far under the 2e-2 gate.
USE_F16G = os.environ.get("KERNEL_F16G", "1") == "1"
# packed fp16 sim: split the exact fp32 Q/K into fp16 hi/lo pairs and
# compute sim = qh*kh + qh*kl + ql*kh as ONE contract-96 fp16 matmul
# (PE: 1 cyc/col vs fp32's 4). Effective ~2^-21 sim precision — measured
# 0 top-4 flips over all 8192 rows (f32r's ~2^-13 flipped enough for
# 0.025 rel err; this is 256x finer).
USE_PACK = os.environ.get("KERNEL_PACK", "1") == "1"

# float32r is *rounded* fp32 (reduced precision) — measured 0.025 rel err
# on this problem, so it stays off; exact fp32 everywhere.
MM_DT = f32r if USE_F32R else f32
# value-path dtype (gather table, gather tiles, mean, output store)
VDT = f16 if USE_F16G else f32
VNP = np.float16 if USE_F16G else np.float32

PAIR_GROUPS = [[0, 1], [2, 3], [4, 5], [6, 7]]

_NC = None


def _emit_warmup(tc, nc):
    from contextlib import ExitStack

    # ~4us of dummy matmuls so the PE p-state ramps to 2.4 GHz while the
    # first input DMA is in flight. Pools scoped so the PSUM bank frees
    # before phase C needs all 8.
    with ExitStack() as wctx:
        wu = wctx.enter_context(tc.tile_pool(name="wu", bufs=1))
        wups = wctx.enter_context(tc.tile_pool(name="wups", bufs=1, space="PSUM"))
        wsb = wu.tile([P, P], f32)
        nc.gpsimd.memset(wsb[:], 1.0)
        wps = wups.tile([P, P], f32)
        for _ in range(10):
            nc.tensor.matmul(wps[:], lhsT=wsb[:], rhs=wsb[:], start=True, stop=True)


def _emit_qk_pack(tc, nc, cpool, qt, kt, tq):
    """Split exact fp32 Q^T/K^T into fp16 hi/lo and build the packed
    contract-96 sim operands qtp=[qh;qh;ql], ktp=[kh;kl;kh] so that
    qtp.T @ ktp = qh*kh + qh*kl + ql*kh (the ll term ~2^-22 is dropped)."""
    qtp = cpool.tile([3 * KQ, tq], f16)
    ktp = cpool.tile([3 * KQ, T], f16)
    nc.scalar.activation(qtp[0:KQ, :], qt[:, 0:tq], IDENT)
    nc.scalar.activation(qtp[KQ : 2 * KQ, :], qt[:, 0:tq], IDENT)
    nc.vector.tensor_sub(qtp[2 * KQ : 3 * KQ, :], qt[:, 0:tq], qtp[0:KQ, :])
    nc.scalar.activation(ktp[0:KQ, :], kt[:], IDENT)
    nc.scalar.activation(ktp[2 * KQ : 3 * KQ, :], kt[:], IDENT)
    nc.vector.tensor_sub(ktp[KQ : 2 * KQ, :], kt[:], ktp[0:KQ, :])
    return qtp, ktp


def _emit_topk_gather(tc, nc, pcd, qt, kt, xg, out):
    """Phases C+D: sim, top-k, gather, mean, store."""
    psim = pcd.enter_context(
        tc.tile_pool(name="psim", bufs=1 if USE_OVL else 2, space="PSUM")
    )
    gpool = pcd.enter_context(tc.tile_pool(name="gpool", bufs=4 if USE_V2CD else 2))
    mpool = pcd.enter_context(tc.tile_pool(name="mpool", bufs=3))
    opool = pcd.enter_context(tc.tile_pool(name="opool", bufs=3 if USE_V2CD else 2))

    for i in range(NT):
        simp = psim.tile([P, T], f32, tag="sim", name=f"sim{i}")
        for c in range(NG):
            nc.tensor.matmul(
                simp[:, ts(c, GT)],
                lhsT=qt[:, ts(i, P)],
                rhs=kt[:, ts(c, GT)],
                start=True,
                stop=True,
            )
        mx = mpool.tile([P, 8], f32, tag="mx", name=f"mx{i}")
        ix = mpool.tile([P, 8], u32, tag="ix", name=f"ix{i}")
        nc.vector.max(out=mx[:], in_=simp[:])
        nc.vector.max_index(out=ix[:], in_max=mx[:], in_values=simp[:])

        if ABLATE == "nogather":
            g = [
                gpool.tile([P, D], VDT, tag=f"g{k}", name=f"g{k}_{i}")
                for k in range(2)
            ]
            nc.gpsimd.memset(g[0][:], 0.5)
            nc.gpsimd.memset(g[1][:], 0.25)
            s01 = opool.tile([P, D], VDT, tag="s01", name=f"s01_{i}")
            nc.vector.tensor_add(s01[:], g[0][:], g[1][:])
        elif USE_V2CD and USE_GIDX2:
            # two fused 2-index gathers: ga[p, j, :] = xg[ix[p, j]] then
            # += xg[ix[p, j+2]] via cce add; one DVE add folds j=0,1.
            ga = gpool.tile([P, 2, D], VDT, tag="ga", name=f"ga_{i}")
            nc.gpsimd.indirect_dma_start(
                out=ga[:],
                out_offset=None,
                in_=xg[:, :],
                in_offset=bass.IndirectOffsetOnAxis(ap=ix[:, 0:2], axis=0),
            )
            nc.gpsimd.indirect_dma_start(
                out=ga[:],
                out_offset=None,
                in_=xg[:, :],
                in_offset=bass.IndirectOffsetOnAxis(ap=ix[:, 2:4], axis=0),
                compute_op=mybir.AluOpType.add,
            )
            s01 = opool.tile([P, D], VDT, tag="s01", name=f"s01_{i}")
            nc.vector.tensor_add(s01[:], ga[:, 0, :], ga[:, 1, :])
        elif USE_CCE:
            g = [
                gpool.tile([P, D], VDT, tag=f"g{k}", name=f"g{k}_{i}")
                for k in range(2)
            ]
            for k in range(KTOP):
                nc.gpsimd.indirect_dma_start(
                    out=g[k % 2][:],
                    out_offset=None,
                    in_=xg[:, :],
                    in_offset=bass.IndirectOffsetOnAxis(ap=ix[:, k : k + 1], axis=0),
                    compute_op=(
                        mybir.AluOpType.add if k >= 2 else mybir.AluOpType.bypass
                    ),
                )
            s01 = opool.tile([P, D], VDT, tag="s01", name=f"s01_{i}")
            nc.vector.tensor_add(s01[:], g[0][:], g[1][:])
        else:
            g = [
                gpool.tile([P, D], VDT, tag=f"g{k}", name=f"g{k}_{i}")
                for k in range(KTOP)
            ]
            for k in range(KTOP):
                nc.gpsimd.indirect_dma_start(
                    out=g[k][:],
                    out_offset=None,
                    in_=xg[:, :],
                    in_offset=bass.IndirectOffsetOnAxis(ap=ix[:, k : k + 1], axis=0),
                )
            s01 = opool.tile([P, D], VDT, tag="s01", name=f"s01_{i}")
            s23 = opool.tile([P, D], VDT, tag="s23", name=f"s23_{i}")
            nc.vector.tensor_add(s01[:], g[0][:], g[1][:])
            nc.vector.tensor_add(s23[:], g[2][:], g[3][:])
            nc.vector.tensor_add(s01[:], s01[:], s23[:])
        # xg rows are pre-scaled by 0.25 on the host (exact power-of-two
        # scale), so s01 already is the 4-neighbor mean. Stores alternate
        # between the ACT and SP HWDGE rings (SP is idle during C/D).
        if USE_OVL:
            # SWDGE store on the Pool queue: issues right after this tile's
            # add (data-ready), never gates the next iteration's SP loads
            # or ACT phase-B work.
            nc.gpsimd.dma_start(out[ts(i, P), :], s01[:])
        elif USE_V2CD:
            seng = nc.scalar if i % 2 == 0 else nc.sync
            seng.dma_start(out[ts(i, P), :], s01[:])
        else:
            nc.sync.dma_start(out[ts(i, P), :], s01[:])


def _emit_pair(tc, nc, xg, xth, wqkt, bqk, out, warmup):
    """Pair-sharing variant: project own T-half only, AllGather K^T.

    Everything is in GLOBAL coordinates: sim columns are global t, the
    gather table xg is the unrolled x[b], and the output rows are the
    core's own global query rows.
    """
    from contextlib import ExitStack

    with ExitStack() as ctx:
        if warmup:
            _emit_warmup(tc, nc)
        cpool = ctx.enter_context(tc.tile_pool(name="consts", bufs=1))
        wq_sb = cpool.tile([P, ND, 2 * KQ], MM_DT)  # [128, 8, 64]; d = dd*128+p
        nc.sync.dma_start(wq_sb[:], wqkt.rearrange("(n p) k -> p n k", p=P))
        bqk_sb = cpool.tile([2 * KQ, 1], f32)
        nc.sync.dma_start(bqk_sb[:], bqk[:])
        qt = cpool.tile([KQ, TQ], f32)  # Q^T (own half) with bias
        kt = cpool.tile([KQ, NGH, TQ], f32)  # K^T (full T) with bias

        dpool = ctx.enter_context(tc.tile_pool(name="ccdram", bufs=1, space="DRAM"))
        cc_in = dpool.tile([KQ, TQ], f32)
        cc_out = dpool.tile([2 * KQ, TQ], f32)

        # ---- phase A: load own xth half + project ----
        with ExitStack() as pa:
            xt_pool = pa.enter_context(tc.tile_pool(name="xt", bufs=3))
            pqkt = pa.enter_context(tc.tile_pool(name="pqkt", bufs=1, space="PSUM"))
            qk_ps = [
                pqkt.tile([2 * KQ, GT], f32, tag=f"qk{c}", name=f"qk_ps{c}")
                for c in range(NGH)
            ]
            kth = cpool.tile([KQ, TQ], f32)  # own biased K^T half
            if ABLATE == "noproj":
                nc.vector.memset(qt[:], 0.001)
                nc.vector.memset(kth[:], 0.002)
            for dd in range(ND if ABLATE != "noproj" else 0):
                xt = xt_pool.tile([P, TQ], MM_DT, tag="xt", name=f"xt{dd}")
                nc.sync.dma_start(xt[:], xth[ts(dd, P), :])
                for c in range(NGH):
                    nc.tensor.matmul(
                        qk_ps[c][:],
                        lhsT=wq_sb[:, dd, :],
                        rhs=xt[:, ts(c, GT)],
                        start=(dd == 0),
                        stop=(dd == ND - 1),
                    )

            # ---- phase B: PSUM -> SBUF with bias ----
            for c in range(NGH if ABLATE != "noproj" else 0):
                nc.scalar.activation(
                    qt[:, ts(c, GT)], qk_ps[c][0:KQ, :], IDENT, bias=bqk_sb[0:KQ, :]
                )
                nc.scalar.activation(
                    kth[:, ts(c, GT)],
                    qk_ps[c][KQ : 2 * KQ, :],
                    IDENT,
                    bias=bqk_sb[KQ : 2 * KQ, :],
                )
        nc.sync.dma_start(cc_in[:], kth[:])
        nc.gpsimd.collective_compute(
            "AllGather",
            mybir.AluOpType.bypass,
            replica_groups=PAIR_GROUPS,
            ins=[cc_in[:]],
            outs=[cc_out[:]],
        )
        # cc_out rows [0:32] = pair rank 0 (global t 0..1023), rows
        # [32:64] = pair rank 1 — global column order for both cores.
        nc.sync.dma_start(kt[:], cc_out.rearrange("(h k) s -> k h s", k=KQ))

        with ExitStack() as pcd:
            ktf = kt.rearrange("k h s -> k (h s)")
            if USE_PACK:
                qt, ktf = _emit_qk_pack(tc, nc, cpool, qt, ktf, TQ)
            _emit_topk_gather(tc, nc, pcd, qt, ktf, xg, out)


def _emit_solo(tc, nc, xg, xrt, wqkt, bqk, out, warmup):
    """Original variant: every core projects all T keys itself (rolled
    coordinates: the core's queries are rows [0:1024) of the rolled x)."""
    from contextlib import ExitStack

    with ExitStack() as ctx:
        if warmup:
            _emit_warmup(tc, nc)
        cpool = ctx.enter_context(tc.tile_pool(name="consts", bufs=1))
        wq_sb = cpool.tile([P, ND, 2 * KQ], MM_DT)
        nc.sync.dma_start(wq_sb[:], wqkt.rearrange("(n p) k -> p n k", p=P))
        bqk_sb = cpool.tile([2 * KQ, 1], f32)
        nc.sync.dma_start(bqk_sb[:], bqk[:])
        qt = cpool.tile([KQ, T], f32)
        kt = cpool.tile([KQ, T], f32)

        with ExitStack() as pa:
            xt_pool = pa.enter_context(tc.tile_pool(name="xt", bufs=3))
            pqkt = pa.enter_context(tc.tile_pool(name="pqkt", bufs=1, space="PSUM"))
            qk_ps = [
                pqkt.tile([2 * KQ, GT], f32, tag=f"qk{c}", name=f"qk_ps{c}")
                for c in range(NG)
            ]
            if ABLATE == "noproj":
                nc.vector.memset(qt[:], 0.001)
                nc.vector.memset(kt[:], 0.002)
            for dd in range(ND if ABLATE != "noproj" else 0):
                xt = xt_pool.tile([P, T], MM_DT, tag="xt", name=f"xt{dd}")
                if USE_OVL:
                    eng = nc.sync  # SP queue only: drains before r+1 needs it
                else:
                    # alternate load issue across both HWDGE rings (SP/ACT)
                    eng = nc.sync if (dd % 2 == 0 or not USE_V2CD) else nc.scalar
                eng.dma_start(xt[:], xrt[ts(dd, P), :])
                for c in range(NG):
                    nc.tensor.matmul(
                        qk_ps[c][:],
                        lhsT=wq_sb[:, dd, :],
                        rhs=xt[:, ts(c, GT)],
                        start=(dd == 0),
                        stop=(dd == ND - 1),
                    )
            for c in range(NG if ABLATE != "noproj" else 0):
                nc.scalar.activation(
                    qt[:, ts(c, GT)], qk_ps[c][0:KQ, :], IDENT, bias=bqk_sb[0:KQ, :]
                )
                nc.scalar.activation(
                    kt[:, ts(c, GT)],
                    qk_ps[c][KQ : 2 * KQ, :],
                    IDENT,
                    bias=bqk_sb[KQ : 2 * KQ, :],
                )

        if USE_PACK:
            qt, kt = _emit_qk_pack(tc, nc, cpool, qt, kt, TQ)
        with ExitStack() as pcd:
            _emit_topk_gather(tc, nc, pcd, qt, kt, xg, out)


def _emit_stream(tc, nc, xg, xrt, wqkt, bqk, out, repeat):
    """All `repeat` iterations emitted into ONE set of shared pools, with
    iteration r+1's phase A/B interleaved between iteration r's C/D tiles.

    Why: (1) closing a tile pool emits a release boundary that any later
    pool reusing the space waits on across all engines — per-iteration
    pool scopes act as inter-iteration barriers (measured ~90us stall).
    (2) Engine queues are strictly in-order, so iteration r+1's phase-A
    matmuls can only fill the PE idle gaps between iteration r's sim
    bursts if they are EMITTED between them. The interleave also keeps
    PE continuously busy so its p-state stays at 2.4 GHz (idle resets
    the clock ramp; cold fp32 matmuls run ~3x slower).

    Schedule: phase A runs group-major (each group's 8-chunk PSUM
    accumulation completes early) split into half-group quads — C/D tile
    i of iteration r is followed by quad (c=i//2, half=i%2) of iteration
    r+1, and by phase B for group c after the second half. All 8 xt
    chunks are SBUF-resident (bufs=2 for cross-iteration rotation).

    PSUM budget: qk_ps 4 banks + sim 4 banks = 8. Queues: loads+consts
    on SP, phase-B activations on ACT, pack subs on Pool (gpsimd) to
    keep them off the DVE scan backbone, sims+A-quads on PE, scans and
    the mean add on DVE, gathers and output stores on Pool/SWDGE.
    """
    from contextlib import ExitStack

    with ExitStack() as ctx:
        _emit_warmup(tc, nc)
        cpool = ctx.enter_context(tc.tile_pool(name="consts", bufs=1))
        wq_sb = cpool.tile([P, ND, 2 * KQ], MM_DT)
        nc.sync.dma_start(wq_sb[:], wqkt.rearrange("(n p) k -> p n k", p=P))
        bqk_sb = cpool.tile([2 * KQ, 1], f32)
        nc.sync.dma_start(bqk_sb[:], bqk[:])

        qkpool = ctx.enter_context(tc.tile_pool(name="qkp", bufs=2))
        xt_pool = ctx.enter_context(tc.tile_pool(name="xt", bufs=2))
        pqkt = ctx.enter_context(tc.tile_pool(name="pqkt", bufs=1, space="PSUM"))
        psim = ctx.enter_context(
            tc.tile_pool(name="psim", bufs=3 if USE_HSIM else 1, space="PSUM")
        )
        # bufs 4/3/3 is the HW-validated config; deepening to 5/4/4
        # (modeled 52.7->50.0) pushed SBUF to ~203/208 KiB/partition and
        # the run died with NRT_EXEC_UNIT_UNRECOVERABLE — not shipped.
        gpool = ctx.enter_context(
            tc.tile_pool(name="gpool", bufs=int(os.environ.get("KERNEL_GBUF", "4")))
        )
        mpool = ctx.enter_context(
            tc.tile_pool(name="mpool", bufs=int(os.environ.get("KERNEL_MBUF", "3")))
        )
        opool = ctx.enter_context(
            tc.tile_pool(name="opool", bufs=int(os.environ.get("KERNEL_OBUF", "3")))
        )
        if N_WARM:
            # keep-warm: always-ready fp16 dummy matmuls into a dedicated
            # PSUM bank fill PE idle gaps so the p-state stays at 2.4 GHz
            # (PE clock decays on idle; cold bursts run ~2-3x slower).
            hotps = ctx.enter_context(tc.tile_pool(name="hotps", bufs=1, space="PSUM"))
            dmy = cpool.tile([P, GT], f16)
            nc.vector.memset(dmy[:], 0.001)
            hot = hotps.tile([P, GT], f32)

            def emit_warm(n):
                for _ in range(n):
                    nc.tensor.matmul(
                        hot[:], lhsT=dmy[:, 0:P], rhs=dmy[:], start=True, stop=True
                    )
        else:
            def emit_warm(n):
                pass

        def alloc_iter(r):
            return dict(
                xts=[
                    xt_pool.tile([P, T], MM_DT, tag=f"xt{dd}", name=f"xt{dd}_{r}")
                    for dd in range(ND)
                ],
                ps=[
                    pqkt.tile([2 * P // 2, GT], f32, tag=f"qk{j}", name=f"qk_ps{j}_{r}")
                    for j in range(NG // 2)
                ],
                qt=qkpool.tile([KQ, TQ], f32, tag="qt", name=f"qt_{r}"),
                kt=qkpool.tile([KQ, T], f32, tag="kt", name=f"kt_{r}"),
                qtp=qkpool.tile([3 * KQ, TQ], f16, tag="qtp", name=f"qtp_{r}"),
                ktp=qkpool.tile([3 * KQ, T], f16, tag="ktp", name=f"ktp_{r}"),
            )

        def emit_loads(it):
            for dd in range(ND):
                nc.sync.dma_start(it["xts"][dd][:], xrt[ts(dd, P), :])

        def emit_A_quad(it, c, half):
            j, sub = divmod(c, 2)
            dst = it["ps"][j][sub * 2 * KQ : (sub + 1) * 2 * KQ, :]
            for dd in range(4 * half, 4 * half + 4):
                nc.tensor.matmul(
                    dst,
                    lhsT=wq_sb[:, dd, :],
                    rhs=it["xts"][dd][:, ts(c, GT)],
                    start=(dd == 0),
                    stop=(dd == ND - 1),
                )

        def emit_B_group(it, c):
            qt, kt, qtp, ktp = it["qt"], it["kt"], it["qtp"], it["ktp"]
            sl = ts(c, GT)
            sub = nc.gpsimd.tensor_sub if USE_GSUB else nc.vector.tensor_sub
            j, psub = divmod(c, 2)
            base = psub * 2 * KQ
            if c < NGH:
                nc.scalar.activation(
                    qt[:, sl],
                    it["ps"][j][base : base + KQ, :],
                    IDENT,
                    bias=bqk_sb[0:KQ, :],
                )
            nc.scalar.activation(
                kt[:, sl],
                it["ps"][j][base + KQ : base + 2 * KQ, :],
                IDENT,
                bias=bqk_sb[KQ : 2 * KQ, :],
            )
            if c < NGH:
                nc.scalar.activation(qtp[0:KQ, sl], qt[:, sl], IDENT)
                nc.scalar.activation(qtp[KQ : 2 * KQ, sl], qt[:, sl], IDENT)
                sub(qtp[2 * KQ : 3 * KQ, sl], qt[:, sl], qtp[0:KQ, sl])
            nc.scalar.activation(ktp[0:KQ, sl], kt[:, sl], IDENT)
            nc.scalar.activation(ktp[2 * KQ : 3 * KQ, sl], kt[:, sl], IDENT)
            sub(ktp[KQ : 2 * KQ, sl], kt[:, sl], ktp[0:KQ, sl])

        def emit_CD_tile(it, r, i):
            qtp, ktp = it["qtp"], it["ktp"]
            if USE_HSIM:
                simh = [
                    psim.tile([P, TH], f32, tag="sim", name=f"sim{i}h{h}_{r}")
                    for h in range(2)
                ]
                for h in range(2):
                    for c in range(2):
                        g = 2 * h + c
                        nc.tensor.matmul(
                            simh[h][:, ts(c, GT)],
                            lhsT=qtp[:, ts(i, P)],
                            rhs=ktp[:, ts(g, GT)],
                            start=True,
                            stop=True,
                        )
                mxh = mpool.tile([P, 2, 8], f32, tag="mxh", name=f"mxh{i}_{r}")
                nc.vector.max(out=mxh[:, 0, :], in_=simh[0][:])
                nc.vector.max(out=mxh[:, 1, :], in_=simh[1][:])
                vm = mpool.tile([P, 8], f32, tag="vm", name=f"vm{i}_{r}")
                nc.vector.max(out=vm[:], in_=mxh[:])
                ixh0 = mpool.tile([P, 8], u32, tag="ixh0", name=f"ixh0_{i}_{r}")
                ixh1 = mpool.tile([P, 8], u32, tag="ixh1", name=f"ixh1_{i}_{r}")
                nc.vector.max_index(out=ixh0[:], in_max=vm[:], in_values=simh[0][:])
                nc.vector.max_index(out=ixh1[:], in_max=vm[:], in_values=simh[1][:])
                ix = mpool.tile([P, 8], u32, tag="ix", name=f"ix{i}_{r}")
                nc.vector.scalar_tensor_tensor(
                    out=ix[:],
                    in0=ixh1[:],
                    scalar=TH,
                    in1=ixh0[:],
                    op0=mybir.AluOpType.add,
                    op1=mybir.AluOpType.min,
                )
            else:
                simp = psim.tile([P, T], f32, tag="sim", name=f"sim{i}_{r}")
                for c in range(NG):
                    nc.tensor.matmul(
                        simp[:, ts(c, GT)],
                        lhsT=qtp[:, ts(i, P)],
                        rhs=ktp[:, ts(c, GT)],
                        start=True,
                        stop=True,
                    )
                mx = mpool.tile([P, 8], f32, tag="mx", name=f"mx{i}_{r}")
                ix = mpool.tile([P, 8], u32, tag="ix", name=f"ix{i}_{r}")
                nc.vector.max(out=mx[:], in_=simp[:])
                nc.vector.max_index(out=ix[:], in_max=mx[:], in_values=simp[:])

            if USE_GIDX2:
                ga = gpool.tile([P, 2, D], VDT, tag="ga", name=f"ga_{i}_{r}")
                nc.gpsimd.indirect_dma_start(
                    out=ga[:],
                    out_offset=None,
                    in_=xg[:, :],
                    in_offset=bass.IndirectOffsetOnAxis(ap=ix[:, 0:2], axis=0),
                )
                nc.gpsimd.indirect_dma_start(
                    out=ga[:],
                    out_offset=None,
                    in_=xg[:, :],
                    in_offset=bass.IndirectOffsetOnAxis(ap=ix[:, 2:4], axis=0),
                    compute_op=mybir.AluOpType.add,
                )
                a0, a1 = ga[:, 0, :], ga[:, 1, :]
                s01 = opool.tile([P, D], VDT, tag="s01", name=f"s01_{i}_{r}")
                nc.vector.tensor_add(s01[:], a0, a1)
                sv = s01[:]
            elif GFLAT == 4:
                gf = gpool.tile([P, 4 * D], VDT, tag="gf", name=f"gf_{i}_{r}")
                nc.gpsimd.indirect_dma_start(
                    out=gf[:],
                    out_offset=None,
                    in_=xg[:, :],
                    in_offset=bass.IndirectOffsetOnAxis(ap=ix[:, 0:4], axis=0),
                )
                s01 = opool.tile([P, D], VDT, tag="s01", name=f"s01_{i}_{r}")
                s23 = opool.tile([P, D], VDT, tag="s23", name=f"s23_{i}_{r}")
                nc.vector.tensor_add(s01[:], gf[:, 0:D], gf[:, D : 2 * D])
                nc.vector.tensor_add(s23[:], gf[:, 2 * D : 3 * D], gf[:, 3 * D :])
                nc.vector.tensor_add(s01[:], s01[:], s23[:])
                sv = s01[:]
            elif GFLAT == 2:
                gf = gpool.tile([P, 2 * D], VDT, tag="gf", name=f"gf_{i}_{r}")
                nc.gpsimd.indirect_dma_start(
                    out=gf[:],
                    out_offset=None,
                    in_=xg[:, :],
                    in_offset=bass.IndirectOffsetOnAxis(ap=ix[:, 0:2], axis=0),
                )
                nc.gpsimd.indirect_dma_start(
                    out=gf[:],
                    out_offset=None,
                    in_=xg[:, :],
                    in_offset=bass.IndirectOffsetOnAxis(ap=ix[:, 2:4], axis=0),
                    compute_op=mybir.AluOpType.add,
                )
                if USE_SADD:
                    nc.gpsimd.dma_start(out[ts(i, P), :], gf[:, 0:D])
                    nc.gpsimd.dma_start(
                        out[ts(i, P), :], gf[:, D:], accum_op=mybir.AluOpType.add
                    )
                    return
                s01 = opool.tile([P, D], VDT, tag="s01", name=f"s01_{i}_{r}")
                nc.vector.tensor_add(s01[:], gf[:, 0:D], gf[:, D:])
                sv = s01[:]
            elif USE_GCHAIN:
                # all four gathers cce-accumulate into ONE tile: the DMA
                # queue orders the adds, no DVE op needed — the host 0.25
                # pre-scale makes the accumulated tile the final mean.
                g0 = gpool.tile([P, D], VDT, tag="g0", name=f"g0_{i}_{r}")
                for k in range(KTOP):
                    nc.gpsimd.indirect_dma_start(
                        out=g0[:],
                        out_offset=None,
                        in_=xg[:, :],
                        in_offset=bass.IndirectOffsetOnAxis(
                            ap=ix[:, k : k + 1], axis=0
                        ),
                        compute_op=(
                            mybir.AluOpType.add if k >= 1 else mybir.AluOpType.bypass
                        ),
                    )
                sv = g0[:]
            else:
                g = [
                    gpool.tile([P, D], VDT, tag=f"g{k}", name=f"g{k}_{i}_{r}")
                    for k in range(2)
                ]
                for k in range(KTOP):
                    nc.gpsimd.indirect_dma_start(
                        out=g[k % 2][:],
                        out_offset=None,
                        in_=xg[:, :],
                        in_offset=bass.IndirectOffsetOnAxis(
                            ap=ix[:, k : k + 1], axis=0
                        ),
                        compute_op=(
                            mybir.AluOpType.add if k >= 2 else mybir.AluOpType.bypass
                        ),
                    )
                if USE_SADD:
                    nc.gpsimd.dma_start(out[ts(i, P), :], g[0][:])
                    nc.gpsimd.dma_start(
                        out[ts(i, P), :], g[1][:], accum_op=mybir.AluOpType.add
                    )
                    return
                s01 = opool.tile([P, D], VDT, tag="s01", name=f"s01_{i}_{r}")
                nc.vector.tensor_add(s01[:], g[0][:], g[1][:])
                sv = s01[:]
            if USE_PSTORE:
                nc.gpsimd.dma_start(out[ts(i, P), :], sv)
            else:
                nc.scalar.dma_start(out[ts(i, P), :], sv)

        # prologue: iteration 0's phase A + B run un-overlapped
        cur = alloc_iter(0)
        emit_loads(cur)
        for c in range(NG):
            emit_A_quad(cur, c, 0)
            emit_A_quad(cur, c, 1)
            emit_B_group(cur, c)
        for r in range(repeat):
            nxt = None
            if r + 1 < repeat:
                nxt = alloc_iter(r + 1)
                emit_loads(nxt)
            for i in range(NT):
                emit_CD_tile(cur, r, i)
                emit_warm(N_WARM)
                if nxt is not None:
                    # front-load the late groups: c2/c3 run doubled slots at
                    # tiles 4-5 (their chunks have all landed by then), so
                    # the full ktp pack completes ~2 tiles before the
                    # boundary instead of at it — the next iteration's sims
                    # were measured stalling ~17us on B(c3).
                    for c, half in QSCHED[i]:
                        emit_A_quad(nxt, c, half)
                        if half == 1:
                            emit_B_group(nxt, c)
            cur = nxt


def _build_module():
    repeat = int(os.environ.get("KERNEL_REPEAT", "1"))
    nc = bacc.Bacc(
        "TRN2", target_bir_lowering=False, debug=False, num_devices=N_CORES
    )
    if USE_PAIR:
        xg = nc.dram_tensor("xg", [T, D], VDT, kind="ExternalInput").ap()
        xth = nc.dram_tensor("xth", [D, TQ], MM_DT, kind="ExternalInput").ap()
        wqkt = nc.dram_tensor("wqkt", [D, 2 * KQ], MM_DT, kind="ExternalInput").ap()
        bqk = nc.dram_tensor("bqk", [2 * KQ, 1], f32, kind="ExternalInput").ap()
        out = nc.dram_tensor("out", [TQ, D], VDT, kind="ExternalOutput").ap()
        with tile.TileContext(nc) as tc:
            for r in range(repeat):
                _emit_pair(tc, nc, xg, xth, wqkt, bqk, out, warmup=(r == 0))
    else:
        xg = nc.dram_tensor("xr", [T, D], VDT, kind="ExternalInput").ap()
        xrt = nc.dram_tensor("xrt", [D, T], MM_DT, kind="ExternalInput").ap()
        wqkt = nc.dram_tensor("wqkt", [D, 2 * KQ], MM_DT, kind="ExternalInput").ap()
        bqk = nc.dram_tensor("bqk", [2 * KQ, 1], f32, kind="ExternalInput").ap()
        out = nc.dram_tensor("out", [TQ, D], VDT, kind="ExternalOutput").ap()
        with tile.TileContext(nc) as tc:
            if USE_OVL:
                _emit_stream(tc, nc, xg, xrt, wqkt, bqk, out, repeat)
            else:
                for r in range(repeat):
                    _emit_solo(tc, nc, xg, xrt, wqkt, bqk, out, warmup=(r == 0))
    nc.compile()
    return nc


def _get_nc():
    global _NC
    if _NC is None:
        _NC = _build_module()
    return _NC


def _make_in_maps(x, Wq, bq, Wk, bk):
    x = np.ascontiguousarray(np.asarray(x, dtype=np.float32))
    wqkt = np.ascontiguousarray(
        np.concatenate(
            [np.asarray(Wq, np.float32).T, np.asarray(Wk, np.float32).T], axis=1
        )
    )
    bqk = np.concatenate(
        [np.asarray(bq, np.float32), np.asarray(bk, np.float32)]
    )[:, None]
    bqk = np.ascontiguousarray(bqk)
    in_maps = []
    # exact (power of two) pre-scale; fp16 value-path cast adds ~4e-4 rel err
    xq = (x * np.float32(0.25)).astype(VNP)
    for c in range(N_CORES):
        b, h = divmod(c, 2)
        off = h * TQ
        xb = x[b]
        if USE_PAIR:
            in_maps.append(
                {
                    "xg": np.ascontiguousarray(xq[b]),
                    "xth": np.ascontiguousarray(xb[off : off + TQ].T),
                    "wqkt": wqkt,
                    "bqk": bqk,
                }
            )
        else:
            xrc = (
                np.concatenate([xq[b][off:], xq[b][:off]], axis=0)
                if off
                else xq[b]
            )
            in_maps.append(
                {
                    "xr": np.ascontiguousarray(xrc),
                    "xrt": np.ascontiguousarray(xb.T) if off == 0 else
                           np.ascontiguousarray(
                               np.concatenate([xb[off:], xb[:off]], axis=0).T),
                    "wqkt": wqkt,
                    "bqk": bqk,
                }
            )
    return in_maps


def run(x, Wq, bq, Wk, bk, trace=False):
    """Run on 8 cores; returns (full_output, BassKernelResults)."""
    in_maps = _make_in_maps(x, Wq, bq, Wk, bk)
    nc = _get_nc()
    res = run_bass_kernel_spmd(nc, in_maps, list(range(N_CORES)), trace=trace)
    outf = np.empty((B, T, D), np.float32)
    for c in range(N_CORES):
        b, h = divmod(c, 2)
        # device stores VDT (fp16); upcast to fp32 on assignment
        outf[b, h * TQ : (h + 1) * TQ] = res.results[c]["out"]
    return outf, res


def kernel(x, Wq, bq, Wk, bk):
    outf, _ = run(x, Wq, bq, Wk, bk, trace=False)
    return outf

